# revision 30
# baseline (speedup 1.0000x reference)
"""ACDC channel-FFT module as a single complex channel-mixing matmul on 8 TRN2 cores.

Math: reference computes out = take(ifft(fft(x*A, ax=1)*D, ax=1) + bias, perm, ax=1)/sqrt(C).
ifft(diag(D) fft(.)) is a circulant linear operator M = circ(ifft(D)) on the channel
axis, so the whole module collapses to
    out[b, i, s] = sum_k W[i, k] * x[b, k, s] + bias[perm[i]]/sqrt(C)
with W = (M[perm, :] * A[None, :]) / sqrt(C)   (complex 1024x1024, host-precomputed).

Device work per core (one batch element): two real fp16 matmuls (Re W, Im W) of
(1024x1024) @ (1024x4096) accumulated in fp32 PSUM, bias folded into the PSUM
eviction, real/imag interleaved on-chip so the DRAM output is directly complex64.
"""

import numpy as np

import concourse.bass as bass
import concourse.mybir as mybir
from concourse import bacc
from concourse.tile import TileContext
from concourse.bass_utils import run_bass_kernel_spmd

B, C, S = 8, 1024, 4096
P = 128
KB = C // P            # contraction blocks
IB = C // P            # output-channel blocks
NCHUNK = 512           # moving free-dim per matmul (one PSUM bank of fp32)
NCH = S // NCHUNK
FSCALE = 256.0         # fp16 weight pre-scale (avoids subnormal weights)
N_CORES = 8

_CACHE = {}


def _build_nc():
    nc = bacc.Bacc()
    # x pre-swizzled on host to [p, sq, kb*512+s'] so each s-chunk DMA moves
    # one contiguous 8 KB segment per partition (128 descriptors total).
    x = nc.dram_tensor("x", [P, NCH, KB * NCHUNK], mybir.dt.float16, kind="ExternalInput")
    # weights pre-swizzled on host: wr[i, p, k*128+m] = Re(W).T[128k+p, 128i+m]
    # so each out-block i only depends on its own 512 KB weight tile.
    wr = nc.dram_tensor("wr", [IB, P, C], mybir.dt.float16, kind="ExternalInput")
    wi = nc.dram_tensor("wi", [IB, P, C], mybir.dt.float16, kind="ExternalInput")
    bias = nc.dram_tensor("bias", [P, IB], mybir.dt.float32, kind="ExternalInput")
    out = nc.dram_tensor("out", [C, 2 * S], mybir.dt.float32, kind="ExternalOutput")

    outr = out.rearrange("(ib p) s2 -> ib p s2", p=P)

    with TileContext(nc) as tc:
        with (
            tc.tile_pool(name="persist", bufs=1) as pp,
            tc.tile_pool(name="outp", bufs=6) as op,
            tc.tile_pool(name="ps", bufs=4, space="PSUM") as ps,
        ):
            # DMA issue order drives readiness. The PE's first accumulation
            # group needs x s-chunk 0 plus only the k=0 slice of out-block-0
            # real weights, so i=0 weights are split per-k into small tiles;
            # later weights interleave with x s-chunks so neither starves.
            xt = [None] * NCH
            wrt, wit = [None] * IB, [None] * IB

            def _load_x_chunk(sq):
                t = pp.tile([P, KB * NCHUNK], mybir.dt.float16, tag=f"x{sq}")
                nc.sync.dma_start(out=t, in_=x[:, sq, :])
                xt[sq] = t

            def _load_w(i):
                twr = pp.tile([P, C], mybir.dt.float16, tag=f"wr{i}")
                nc.gpsimd.dma_start(out=twr, in_=wr[i])
                wrt[i] = twr
                twi = pp.tile([P, C], mybir.dt.float16, tag=f"wi{i}")
                nc.gpsimd.dma_start(out=twi, in_=wi[i])
                wit[i] = twi

            _load_x_chunk(0)
            _load_w(0)
            bt = pp.tile([P, IB], mybir.dt.float32, tag="bias")
            nc.gpsimd.dma_start(out=bt, in_=bias[:, :])
            _load_x_chunk(1)
            _load_w(1)
            _load_x_chunk(2)
            for i in range(2, IB):
                _load_w(i)
            for sq in range(3, NCH):
                _load_x_chunk(sq)

            def _group(sq, i):
                pr = ps.tile([P, NCHUNK], mybir.dt.float32, tag="pr")
                pi = ps.tile([P, NCHUNK], mybir.dt.float32, tag="pi")
                for k in range(KB):
                    nc.tensor.matmul(
                        pr,
                        lhsT=wrt[i][:, bass.ts(k, P)],
                        rhs=xt[sq][:, bass.ts(k, NCHUNK)],
                        start=(k == 0),
                        stop=(k == KB - 1),
                    )
                for k in range(KB):
                    nc.tensor.matmul(
                        pi,
                        lhsT=wit[i][:, bass.ts(k, P)],
                        rhs=xt[sq][:, bass.ts(k, NCHUNK)],
                        start=(k == 0),
                        stop=(k == KB - 1),
                    )
                ot = op.tile([P, 2 * NCHUNK], mybir.dt.float32, tag="ot")
                nc.scalar.activation(
                    ot[:, ::2],
                    pr,
                    mybir.ActivationFunctionType.Identity,
                    bias=bt[:, i : i + 1],
                    scale=1.0 / FSCALE,
                )
                nc.vector.tensor_scalar_mul(ot[:, 1::2], pi, 1.0 / FSCALE)
                if sq == NCH - 1:
                    # tail: split the final transfers across two queues so the
                    # last 512 KB doesn't ride a single ~57 GB/s DMA engine
                    base = sq * 2 * NCHUNK
                    nc.gpsimd.dma_start(
                        out=outr[i][:, bass.ds(base, NCHUNK)], in_=ot[:, :NCHUNK]
                    )
                    nc.scalar.dma_start(
                        out=outr[i][:, bass.ds(base + NCHUNK, NCHUNK)],
                        in_=ot[:, NCHUNK:],
                    )
                else:
                    nc.gpsimd.dma_start(
                        out=outr[i][:, bass.ts(sq, 2 * NCHUNK)], in_=ot
                    )

            # Phase 1 (sq 0-2) runs i-outer so each freshly-arrived weight
            # tile is amortized over 3 s-chunks of PE work — keeps the PE fed
            # while the 4 MB of weights stream in. Phase 2 has everything
            # resident and runs sq-outer.
            for i in range(IB):
                for sq in range(3):
                    _group(sq, i)
            for sq in range(3, NCH):
                for i in range(IB):
                    _group(sq, i)
    nc.compile()
    return nc


def _get_nc():
    if "nc" not in _CACHE:
        _CACHE["nc"] = _build_nc()
    return _CACHE["nc"]


def _host_prep(x, A, D, bias, perm):
    x = np.asarray(x, dtype=np.float32)
    A = np.asarray(A, dtype=np.float64)
    D = np.asarray(D, dtype=np.float64)
    bias = np.asarray(bias, dtype=np.float64)
    perm = np.asarray(perm).astype(np.int64)

    c = np.fft.ifft(D)                                   # circulant kernel of F^-1 diag(D) F
    idx = (np.arange(C)[:, None] - np.arange(C)[None, :]) % C
    M = c[idx]                                           # M[j, k] = c[(j-k) mod C]
    W = M[perm] * A[None, :] / np.sqrt(C)                # (out, in) complex
    Wt = W.T                                             # lhsT layout [k, m]

    def _swz(a):
        # [k*128+p, i*128+m] -> [i, p, k*128+m]
        t = (a * FSCALE).astype(np.float16)
        return np.ascontiguousarray(
            t.reshape(KB, P, IB, P).transpose(2, 1, 0, 3).reshape(IB, P, C)
        )

    wr16 = _swz(Wt.real)
    wi16 = _swz(Wt.imag)
    bias_p = np.ascontiguousarray(
        (bias[perm] / np.sqrt(C)).astype(np.float32).reshape(IB, P).T
    )
    # [b, kb*128+p, sq*512+s'] -> [b, p, sq, kb*512+s']
    x16 = np.ascontiguousarray(
        x.astype(np.float16)
        .reshape(B, KB, P, NCH, NCHUNK)
        .transpose(0, 2, 3, 1, 4)
        .reshape(B, P, NCH, KB * NCHUNK)
    )
    return x16, wr16, wi16, bias_p


def _run(x, A, D, bias, perm, trace=False):
    x16, wr16, wi16, bias_p = _host_prep(x, A, D, bias, perm)
    nc = _get_nc()
    in_maps = [
        {"x": x16[i], "wr": wr16, "wi": wi16, "bias": bias_p} for i in range(N_CORES)
    ]
    res = run_bass_kernel_spmd(nc, in_maps, core_ids=list(range(N_CORES)), trace=trace)
    outs = [np.asarray(res.results[i]["out"]) for i in range(N_CORES)]
    full = np.stack(outs, axis=0).reshape(B, C, S, 2)
    return np.ascontiguousarray(full).view(np.complex64).reshape(B, C, S), res


def kernel(x, A, D, bias, perm):
    out, _ = _run(x, A, D, bias, perm, trace=False)
    return out
